# revision 18
# baseline (speedup 1.0000x reference)
"""Trainium2 Bass kernel for 5-relation GAT (nn_GAT_76716705841462). v2.

Strategy: destination-sharded, collective-free, bf16, host-built one-hots.
  * Host prep: 128-dst windows dealt across 8 cores by sorted edge count
    (equalizes per-slot block counts; SPMD program shared across cores).
    Per (window, rel): non-self-loop edges split lo (src<32768) / hi
    (src>=17280) for int16 gather indices; block counts = max over cores.
    Host precomputes one-hot O [slot e, dst n] and OT [dst n, slot e]
    (bf16); pad slots are zero rows so no masking is needed on device.
  * Phase A (replicated): node table T[n] = [h0|1|h1|1|as(2)|ad(2)|pad]
    (384 bf16 = 768 B rows) via bf16 matmul; plus per-core Twin table
    (same layout) for the core's dealt windows from xT_loc.
  * Phase B per window: 2 dma_gathers (lo/hi, rels concat) of 768-B rows;
    a_dst per edge slot via one tiny PE matmul per block (OT_b.T @ ad_win);
    expl = exp(leakyrelu(a_src + a_dst)) on [128, 2*nblk] tiles; weighted
    one-hot wt_bh = O_b * expl_bh built on the Scalar engine (Copy
    activation, per-partition scale); TensorE accumulates wt.T @ [G_h|1]
    into ps_r [128, 258] over the rel's blocks. Self-loops are handled
    densely once per window (shared by all 5 rels: same logit, h row in
    Twin). Normalize per rel, accumulate, add 5*bias.
"""

import numpy as np
import ml_dtypes

import concourse.bacc as bacc
import concourse.bass as bass
import concourse.mybir as mybir
import concourse.tile as tile
from concourse.library_config import mlp

P = 128
H = 2
C = 128
D = 256
R = 5
TW = 384          # table row width in bf16 elements (768 B, %256 == 0)
A_OFF = 258       # a_src at 258:260, a_dst at 260:262
LOW_CAP = 32768
NEG = 0.2
EPS = 1e-16

f32 = mybir.dt.float32
bf16 = mybir.dt.bfloat16
i16 = mybir.dt.int16
BF = ml_dtypes.bfloat16

_CACHE = {}
_RUN_KWARGS = {}
_LAST_RESULT = None


def build_program(n_tiles, t_rows, w_pc, h0, blos, bhis, num_devices):
    """blos/bhis: per window j, list of R ints (lo/hi block counts)."""
    import os
    ablate = set(os.environ.get("K_ABLATE", "").split(","))
    wincap = int(os.environ.get("K_WINCAP", 10**9))
    tilecap = int(os.environ.get("K_TILECAP", 10**9))
    nc = bacc.Bacc("TRN2", target_bir_lowering=False, debug=False,
                   num_devices=num_devices)

    nw_p = n_tiles * P
    xT = nc.dram_tensor("xT", [D, nw_p], bf16, kind="ExternalInput")
    xT_loc = nc.dram_tensor("xT_loc", [D, w_pc * P], bf16,
                            kind="ExternalInput")
    Wsrc = nc.dram_tensor("Wsrc", [D, D], f32, kind="ExternalInput")
    Wdst = nc.dram_tensor("Wdst", [D, D], f32, kind="ExternalInput")
    atts = nc.dram_tensor("atts", [1, D], f32, kind="ExternalInput")
    attd = nc.dram_tensor("attd", [1, D], f32, kind="ExternalInput")
    bias_in = nc.dram_tensor("bias_in", [1, D], f32, kind="ExternalInput")

    nblo = [sum(b) for b in blos]
    nbhi = [sum(b) for b in bhis]
    nblk = [a + b for a, b in zip(nblo, nbhi)]
    lo_cols = [n * P // 16 for n in nblo]
    hi_cols = [n * P // 16 for n in nbhi]
    lo_off = np.concatenate([[0], np.cumsum(lo_cols)]).astype(int)
    hi_off = np.concatenate([[0], np.cumsum(hi_cols)]).astype(int)
    ob_off = np.concatenate([[0], np.cumsum([n * P for n in nblk])]).astype(int)

    loidx = nc.dram_tensor("loidx", [P, int(lo_off[-1])], i16,
                           kind="ExternalInput")
    hiidx = nc.dram_tensor("hiidx", [P, int(hi_off[-1])], i16,
                           kind="ExternalInput")
    Obuf = nc.dram_tensor("Obuf", [P, int(ob_off[-1])], bf16,
                          kind="ExternalInput")
    OTbuf = nc.dram_tensor("OTbuf", [P, int(ob_off[-1])], bf16,
                           kind="ExternalInput")
    y = nc.dram_tensor("y", [w_pc * P, D], f32, kind="ExternalOutput")

    T = nc.dram_tensor("T", [t_rows, TW], bf16)
    Twin = nc.dram_tensor("Twin", [w_pc * P, TW], bf16)

    # ---- TileContext 1: table build ----
    with tile.TileContext(nc) as tc:
        with (
            tc.tile_pool(name="setup", bufs=1) as su,
            tc.tile_pool(name="ps_su", bufs=1, space="PSUM") as psu,
        ):
            ws_h = [su.tile([P, D], f32, name=f"ws_h{k}", tag=f"ws_h{k}")
                    for k in range(2)]
            wd_h = [su.tile([P, D], f32, name=f"wd_h{k}", tag=f"wd_h{k}")
                    for k in range(2)]
            for k in range(2):
                nc.sync.dma_start(ws_h[k][:], Wsrc[k * P:(k + 1) * P, :])
                nc.sync.dma_start(wd_h[k][:], Wdst[k * P:(k + 1) * P, :])
            ones1 = su.tile([1, P], f32, tag="ones1")
            nc.vector.memset(ones1[:], 1.0)
            atts_sb = su.tile([1, D], f32, tag="atts_sb")
            attd_sb = su.tile([1, D], f32, tag="attd_sb")
            nc.sync.dma_start(atts_sb[:], atts[:])
            nc.sync.dma_start(attd_sb[:], attd[:])
            atts_bc = su.tile([P, D], f32, tag="atts_bc")
            attd_bc = su.tile([P, D], f32, tag="attd_bc")
            for row_sb, bc in ((atts_sb, atts_bc), (attd_sb, attd_bc)):
                ps_bc = psu.tile([P, D], f32, name="ps_bc", tag="ps_bc",
                                 bufs=2)
                nc.tensor.matmul(out=ps_bc[:], lhsT=ones1[:], rhs=row_sb[:],
                                 start=True, stop=True)
                nc.vector.tensor_copy(bc[:], ps_bc[:])

            rhs_kf = [su.tile([P, TW], f32, name=f"rhs_kf{k}", tag=f"rhs_kf{k}")
                      for k in range(2)]
            rhs_k = [su.tile([P, TW], bf16, name=f"rhs_k{k}", tag=f"rhs_k{k}")
                     for k in range(2)]
            for k in range(2):
                rk = rhs_kf[k]
                nc.vector.memset(rk[:], 0.0)
                nc.vector.tensor_copy(rk[:, 0:C], ws_h[k][:, 0:C])
                nc.vector.tensor_copy(rk[:, C + 1:2 * C + 1], ws_h[k][:, C:D])
                for h in range(H):
                    for src_w, src_bc, col in (
                        (ws_h[k], atts_bc, A_OFF + h),
                        (wd_h[k], attd_bc, A_OFF + 2 + h),
                    ):
                        scratch = su.tile([P, C], f32, name="vscr",
                                          tag="vscr", bufs=2)
                        nc.vector.tensor_tensor(
                            out=scratch[:],
                            in0=src_w[:, h * C:(h + 1) * C],
                            in1=src_bc[:, h * C:(h + 1) * C],
                            op=mybir.AluOpType.mult)
                        nc.vector.tensor_reduce(
                            out=rk[:, col:col + 1], in_=scratch[:],
                            axis=mybir.AxisListType.X,
                            op=mybir.AluOpType.add)
                nc.vector.tensor_copy(rhs_k[k][:], rk[:])

            with (
                tc.tile_pool(name="sb_tbl", bufs=3) as stp,
                tc.tile_pool(name="ps_tbl", bufs=4, space="PSUM") as ptp,
            ):
                # pair-tile batching: 2 node tiles per iteration; input loads
                # dispatched from the scalar engine's HWDGE ring, the table
                # write as a single two-chunk DMA from the sync ring.
                t_list = (list(range(min(n_tiles, tilecap)))
                          + list(range(n_tiles, n_tiles + w_pc)))
                groups = []
                i = 0
                while i < len(t_list):
                    if (i + 1 < len(t_list)
                            and t_list[i + 1] == t_list[i] + 1
                            and (t_list[i] < n_tiles) == (t_list[i + 1] < n_tiles)):
                        groups.append((t_list[i], 2))
                        i += 2
                    else:
                        groups.append((t_list[i], 1))
                        i += 1
                for t, gn in groups:
                    if t < n_tiles:
                        src, dst, row0 = xT, T, t * P
                    else:
                        src, dst, row0 = xT_loc, Twin, (t - n_tiles) * P
                    xk0 = stp.tile([P, gn * P], bf16, name="xk0", tag="xk0")
                    xk1 = stp.tile([P, gn * P], bf16, name="xk1", tag="xk1")
                    nc.scalar.dma_start(xk0[:], src[0:P, row0:row0 + gn * P])
                    nc.scalar.dma_start(xk1[:], src[P:D, row0:row0 + gn * P])
                    stg = stp.tile([P, gn * TW], bf16, name="stg", tag="stg")
                    for u in range(gn):
                        ps_t = ptp.tile([P, TW], f32, name="ps_t", tag="ps_t")
                        nc.tensor.matmul(
                            out=ps_t[:], lhsT=xk0[:, u * P:(u + 1) * P],
                            rhs=rhs_k[0][:], start=True, stop=False)
                        nc.tensor.matmul(
                            out=ps_t[:], lhsT=xk1[:, u * P:(u + 1) * P],
                            rhs=rhs_k[1][:], start=False, stop=True)
                        nc.vector.tensor_copy(
                            stg[:, u * TW:(u + 1) * TW], ps_t[:])
                        nc.vector.memset(
                            stg[:, u * TW + C:u * TW + C + 1], 1.0)
                        nc.vector.memset(
                            stg[:, u * TW + 2 * C + 1:u * TW + 2 * C + 2], 1.0)
                    nc.sync.dma_start(
                        dst[row0:row0 + gn * P, :].rearrange(
                            "(u p) c -> p u c", u=gn),
                        stg[:].rearrange("p (u c) -> p u c", u=gn))

    # ---- TileContext 2: attention + aggregation ----
    with tile.TileContext(nc) as tc:
        with (
            tc.tile_pool(name="su2", bufs=1) as su,
            tc.tile_pool(name="ps_su2", bufs=1, space="PSUM") as psu,
            tc.tile_pool(name="sb_g", bufs=3) as sgp,
            tc.tile_pool(name="sb_o", bufs=3) as sop_,
            tc.tile_pool(name="sb_idx", bufs=3) as sip,
            tc.tile_pool(name="sb_wt", bufs=4) as swp,
            tc.tile_pool(name="sb_sm", bufs=3) as ssp,
            tc.tile_pool(name="sb_out", bufs=2) as sout,
            tc.tile_pool(name="ps_mm", bufs=2, space="PSUM") as pmp,
            tc.tile_pool(name="ps_ad", bufs=2, space="PSUM") as pap,
        ):
            nc.gpsimd.load_library(mlp)
            bias_sb = su.tile([1, D], f32, tag="bias_sb")
            nc.sync.dma_start(bias_sb[:], bias_in[:])
            ones1 = su.tile([1, P], f32, tag="ones1b")
            nc.vector.memset(ones1[:], 1.0)
            bias5 = su.tile([P, D], f32, tag="bias5")
            ps_bc = psu.tile([P, D], f32, tag="ps_bc2")
            nc.tensor.matmul(out=ps_bc[:], lhsT=ones1[:], rhs=bias_sb[:],
                             start=True, stop=True)
            nc.vector.tensor_scalar_mul(bias5[:], ps_bc[:], float(R))

            for j in range(min(w_pc, wincap)):
                rows = slice(j * P, (j + 1) * P)
                nb, nl, nh = nblk[j], nblo[j], nbhi[j]
                lo_b0 = np.concatenate([[0], np.cumsum(blos[j])]).astype(int)
                hi_b0 = np.concatenate([[0], np.cumsum(bhis[j])]).astype(int)

                twin = sgp.tile([P, TW], bf16, name="twin", tag="twin")
                nc.sync.dma_start(twin[:], Twin[rows, :])
                lo_t = sip.tile([P, max(lo_cols[j], 16)], i16, name="lo_t",
                                tag="lo_t")
                hi_t = sip.tile([P, max(hi_cols[j], 16)], i16, name="hi_t",
                                tag="hi_t")
                if lo_cols[j]:
                    nc.sync.dma_start(
                        lo_t[:, :lo_cols[j]],
                        loidx[:, int(lo_off[j]):int(lo_off[j + 1])])
                if hi_cols[j]:
                    nc.sync.dma_start(
                        hi_t[:, :hi_cols[j]],
                        hiidx[:, int(hi_off[j]):int(hi_off[j + 1])])
                Ot = sop_.tile([P, nb * P], bf16, name="Ot", tag="Ot")
                OTt = sop_.tile([P, nb * P], bf16, name="OTt", tag="OTt")
                nc.scalar.dma_start(
                    Ot[:], Obuf[:, int(ob_off[j]):int(ob_off[j + 1])])
                nc.scalar.dma_start(
                    OTt[:], OTbuf[:, int(ob_off[j]):int(ob_off[j + 1])])

                G = sgp.tile([P, nb * TW], bf16, name="G", tag="G")
                if "nogather" in ablate:
                    nc.vector.memset(G[:], 0.25)
                else:
                    # dma_gather is capped at 1024 indices (8 blocks) per call
                    for b0 in range(0, nl, 8):
                        bn = min(8, nl - b0)
                        nc.gpsimd.dma_gather(
                            out_ap=G[:, b0 * TW:(b0 + bn) * TW].rearrange(
                                "p (b e) -> p b e", e=TW),
                            in_ap=T[0:LOW_CAP, :],
                            idxs_ap=lo_t[:, b0 * 8:(b0 + bn) * 8],
                            num_idxs=bn * P, num_idxs_reg=bn * P,
                            elem_size=TW)
                    for b0 in range(0, nh, 8):
                        bn = min(8, nh - b0)
                        nc.gpsimd.dma_gather(
                            out_ap=G[:, (nl + b0) * TW:(nl + b0 + bn) * TW]
                                .rearrange("p (b e) -> p b e", e=TW),
                            in_ap=T[h0:t_rows, :],
                            idxs_ap=hi_t[:, b0 * 8:(b0 + bn) * 8],
                            num_idxs=bn * P, num_idxs_reg=bn * P,
                            elem_size=TW)

                # a_dst per edge slot: per block, [128e, 2] = OT_b.T @ ad_win
                ps_ad = pap.tile([P, 2 * nb], f32, name="ps_ad", tag="ps_ad")
                if "noad" in ablate:
                    nc.vector.memset(ps_ad[:], 0.0)
                else:
                    for b in range(nb):
                        nc.tensor.matmul(
                            out=ps_ad[:, 2 * b:2 * b + 2],
                            lhsT=OTt[:, b * P:(b + 1) * P],
                            rhs=twin[:, A_OFF + 2:A_OFF + 4],
                            start=True, stop=True)

                # asum[e, (b h)] = a_src(from G) + a_dst(ps_ad)
                asum = ssp.tile([P, 2 * nb], f32, name="asum", tag="asum")
                nc.vector.tensor_tensor(
                    out=asum[:].rearrange("p (b h) -> p b h", h=2),
                    in0=G[:].rearrange("p (b e) -> p b e", e=TW)
                        [:, :, A_OFF:A_OFF + 2],
                    in1=ps_ad[:].rearrange("p (b h) -> p b h", h=2),
                    op=mybir.AluOpType.add)
                lrl = ssp.tile([P, 2 * nb], f32, name="lrl", tag="lrl")
                nc.vector.scalar_tensor_tensor(
                    out=lrl[:], in0=asum[:], scalar=NEG, in1=asum[:],
                    op0=mybir.AluOpType.mult, op1=mybir.AluOpType.max)
                expl = ssp.tile([P, 2 * nb], f32, name="expl", tag="expl")
                nc.scalar.activation(expl[:], lrl[:],
                                     mybir.ActivationFunctionType.Exp)

                # self-loop terms (shared across rels)
                aslf = ssp.tile([P, 2], f32, name="aslf", tag="aslf")
                nc.vector.tensor_tensor(
                    out=aslf[:], in0=twin[:, A_OFF:A_OFF + 2],
                    in1=twin[:, A_OFF + 2:A_OFF + 4], op=mybir.AluOpType.add)
                lslf = ssp.tile([P, 2], f32, name="lslf", tag="lslf")
                nc.vector.scalar_tensor_tensor(
                    out=lslf[:], in0=aslf[:], scalar=NEG, in1=aslf[:],
                    op0=mybir.AluOpType.mult, op1=mybir.AluOpType.max)
                esl = ssp.tile([P, 2], f32, name="esl", tag="esl")
                nc.scalar.activation(esl[:], lslf[:],
                                     mybir.ActivationFunctionType.Exp)
                Cslf = ssp.tile([P, 2 * C], f32, name="Cslf", tag="Cslf")
                for h in range(H):
                    nc.vector.tensor_scalar_mul(
                        Cslf[:, h * C:(h + 1) * C],
                        twin[:, h * (C + 1):h * (C + 1) + C],
                        esl[:, h:h + 1])

                outacc = sout.tile([P, D], f32, name="outacc", tag="outacc")

                for r in range(R):
                    blk = ([b for b in range(lo_b0[r], lo_b0[r + 1])]
                           + [nl + b for b in range(hi_b0[r], hi_b0[r + 1])])
                    ps = pmp.tile([P, 2 * (C + 1)], f32, name="ps", tag="ps")
                    if "nomm" in ablate:
                        nc.vector.memset(ps[:], 1.0)
                    else:
                        for h in range(H):
                            for bi, b in enumerate(blk):
                                wt = swp.tile([P, P], bf16, name="wt", tag="wt")
                                if "nowt" in ablate:
                                    nc.vector.memset(wt[:], 0.5)
                                elif (b + h) % 2 == 0:
                                    nc.scalar.activation(
                                        wt[:], Ot[:, b * P:(b + 1) * P],
                                        mybir.ActivationFunctionType.Copy,
                                        scale=expl[:, 2 * b + h:2 * b + h + 1])
                                else:
                                    nc.vector.tensor_scalar_mul(
                                        wt[:], Ot[:, b * P:(b + 1) * P],
                                        expl[:, 2 * b + h:2 * b + h + 1])
                                nc.tensor.matmul(
                                    out=ps[:, h * (C + 1):(h + 1) * (C + 1)],
                                    lhsT=wt[:],
                                    rhs=G[:].rearrange("p (b e) -> p b e", e=TW)
                                        [:, b, h * (C + 1):(h + 1) * (C + 1)],
                                    start=(bi == 0), stop=(bi == len(blk) - 1))
                    den = ssp.tile([P, 2], f32, name="den", tag="den")
                    nc.vector.scalar_tensor_tensor(
                        out=den[:],
                        in0=ps[:].rearrange("p (h q) -> p h q", q=C + 1)
                            [:, :, C:C + 1].rearrange("p h o -> p (h o)"),
                        scalar=EPS, in1=esl[:],
                        op0=mybir.AluOpType.add, op1=mybir.AluOpType.add)
                    recip = ssp.tile([P, 2], f32, name="recip", tag="recip")
                    nc.vector.reciprocal(recip[:], den[:])
                    num = ssp.tile([P, 2 * C], f32, name="num", tag="num")
                    nc.vector.tensor_tensor(
                        out=num[:].rearrange("p (h c) -> p h c", h=2),
                        in0=ps[:].rearrange("p (h q) -> p h q", q=C + 1)
                            [:, :, 0:C],
                        in1=Cslf[:].rearrange("p (h c) -> p h c", h=2),
                        op=mybir.AluOpType.add)
                    for h in range(H):
                        osl = outacc[:, h * C:(h + 1) * C]
                        nsl = num[:, h * C:(h + 1) * C]
                        if r == 0:
                            nc.vector.tensor_scalar_mul(
                                osl, nsl, recip[:, h:h + 1])
                        else:
                            nc.vector.scalar_tensor_tensor(
                                out=osl, in0=nsl, scalar=recip[:, h:h + 1],
                                in1=osl, op0=mybir.AluOpType.mult,
                                op1=mybir.AluOpType.add)
                nc.vector.tensor_tensor(out=outacc[:], in0=outacc[:],
                                        in1=bias5[:], op=mybir.AluOpType.add)
                nc.sync.dma_start(y[rows, :], outacc[:])

    nc.finalize()
    return nc


def _wrap16(vals):
    """[n] int array -> 16-partition-wrapped [128, n//16] int16 (replicated)."""
    n = len(vals)
    assert n % 16 == 0
    a = np.asarray(vals, np.int16).reshape(n // 16, 16).T
    return np.tile(a, (8, 1))


def prep_inputs(inputs, ncores):
    x = np.asarray(inputs["x"], dtype=np.float32)
    N = x.shape[0]
    nw_real = -(-N // P)
    NW = -(-nw_real // ncores) * ncores
    w_pc = NW // ncores
    n_tiles = nw_real
    t_rows = n_tiles * P
    h0 = t_rows - LOW_CAP

    rels = ["parent", "child", "precede", "follow", "peer"]
    ebuckets = [[None] * NW for _ in range(R)]
    totals = np.zeros(NW, np.int64)
    for r, rn in enumerate(rels):
        ei = np.asarray(inputs[f"edge_index_{rn}"])
        src = ei[0].astype(np.int64)
        dst = ei[1].astype(np.int64)
        order = np.argsort(dst, kind="stable")
        src, dst = src[order], dst[order]
        w_of = dst // P
        cnt = np.bincount(w_of, minlength=NW)
        starts = np.zeros(NW + 1, np.int64)
        np.cumsum(cnt, out=starts[1:])
        for w in range(NW):
            s, e = starts[w], starts[w + 1]
            ebuckets[r][w] = (src[s:e], dst[s:e] - w * P)
            totals[w] += e - s

    order = np.argsort(-totals, kind="stable")
    perm = np.zeros((ncores, w_pc), np.int64)
    for j in range(w_pc):
        grp = order[j * ncores:(j + 1) * ncores]
        if j % 2:
            grp = grp[::-1]
        perm[:, j] = grp

    blos, bhis = [], []
    asn = {}
    for j in range(w_pc):
        blo_j, bhi_j = [], []
        for r in range(R):
            must_lo = np.zeros(ncores, np.int64)
            must_hi = np.zeros(ncores, np.int64)
            tot = np.zeros(ncores, np.int64)
            for c in range(ncores):
                src, _ = ebuckets[r][perm[c, j]]
                must_lo[c] = int((src < h0).sum())
                must_hi[c] = int((src >= LOW_CAP).sum())
                tot[c] = len(src)
            BT = max(1, int(-(-tot.max() // P)))
            B1 = int(-(-must_lo.max() // P))
            B2 = BT - B1
            if B2 * P < must_hi.max():
                B2 = int(-(-must_hi.max() // P))
                B1 = BT - B2
                if B1 * P < must_lo.max():
                    BT += 1
                    B1 = BT - B2
            assert B1 * P >= must_lo.max() and B2 * P >= must_hi.max(), (
                j, r, B1, B2, must_lo.max(), must_hi.max())
            blo_j.append(B1)
            bhi_j.append(B2)
            for c in range(ncores):
                src, dl = ebuckets[r][perm[c, j]]
                is_lo = src < h0
                is_hi = src >= LOW_CAP
                flex = ~is_lo & ~is_hi
                n_lo = min(B1 * P, len(src) - int(is_hi.sum()))
                fi = np.flatnonzero(flex)
                n_flex_lo = n_lo - int(is_lo.sum())
                lo_sel = np.concatenate(
                    [np.flatnonzero(is_lo), fi[:n_flex_lo]])
                hi_sel = np.concatenate(
                    [np.flatnonzero(is_hi), fi[n_flex_lo:]])
                assert len(lo_sel) == n_lo
                assert len(hi_sel) == len(src) - n_lo <= B2 * P
                lo_src = np.zeros(B1 * P, np.int64)
                hi_src = np.zeros(B2 * P, np.int64)
                lo_dst = np.full(B1 * P, -1, np.int64)
                hi_dst = np.full(B2 * P, -1, np.int64)
                lo_src[:len(lo_sel)] = src[lo_sel]
                hi_src[:len(hi_sel)] = src[hi_sel] - h0
                lo_dst[:len(lo_sel)] = dl[lo_sel]
                hi_dst[:len(hi_sel)] = dl[hi_sel]
                asn[(c, j, r)] = (lo_src, hi_src, lo_dst, hi_dst)
        blos.append(blo_j)
        bhis.append(bhi_j)

    xTf = np.zeros((D, max(t_rows, NW * P)), np.float32)
    xTf[:, :N] = x.T
    xT_bw = xTf.astype(BF)
    xT_b = np.ascontiguousarray(xT_bw[:, :t_rows])

    shared = {
        "xT": xT_b,
        "Wsrc": np.ascontiguousarray(np.asarray(inputs["W_src"], np.float32)),
        "Wdst": np.ascontiguousarray(np.asarray(inputs["W_dst"], np.float32)),
        "atts": np.asarray(inputs["att_src"], np.float32).reshape(1, D).copy(),
        "attd": np.asarray(inputs["att_dst"], np.float32).reshape(1, D).copy(),
        "bias_in": np.asarray(inputs["bias"], np.float32).reshape(1, D).copy(),
    }

    nblo = [sum(b) for b in blos]
    nbhi = [sum(b) for b in bhis]
    nblk = [a + b for a, b in zip(nblo, nbhi)]
    lo_colsT = sum(n * P // 16 for n in nblo)
    hi_colsT = sum(n * P // 16 for n in nbhi)
    ob_colsT = sum(n * P for n in nblk)

    eye = np.eye(P, dtype=BF)
    percore = []
    for c in range(ncores):
        loidx = np.zeros((P, lo_colsT), np.int16)
        hiidx = np.zeros((P, hi_colsT), np.int16)
        Obuf = np.zeros((P, ob_colsT), BF)
        OTbuf = np.zeros((P, ob_colsT), BF)
        lo_p = hi_p = ob_p = 0
        for j in range(w_pc):
            lo_all = [asn[(c, j, r)][0] for r in range(R)]
            hi_all = [asn[(c, j, r)][1] for r in range(R)]
            dst_all = ([asn[(c, j, r)][2] for r in range(R)]
                       + [asn[(c, j, r)][3] for r in range(R)])
            lo_cat = np.concatenate(lo_all)
            hi_cat = np.concatenate(hi_all)
            dst_cat = np.concatenate(dst_all)
            nl, nh = len(lo_cat) // P, len(hi_cat) // P
            if nl:
                loidx[:, lo_p:lo_p + nl * P // 16] = _wrap16(lo_cat)
            if nh:
                hiidx[:, hi_p:hi_p + nh * P // 16] = _wrap16(hi_cat)
            lo_p += nl * P // 16
            hi_p += nh * P // 16
            nb = nl + nh
            dst_slots = dst_cat.reshape(nb, P)
            for b in range(nb):
                d = dst_slots[b]
                val = np.zeros((P, P), BF)
                valid = d >= 0
                val[valid, :] = eye[d[valid], :]
                Obuf[:, ob_p + b * P:ob_p + (b + 1) * P] = val
                OTbuf[:, ob_p + b * P:ob_p + (b + 1) * P] = val.T
            ob_p += nb * P
        cols = np.concatenate(
            [np.arange(perm[c, j] * P, (perm[c, j] + 1) * P)
             for j in range(w_pc)])
        percore.append({
            "loidx": loidx, "hiidx": hiidx, "Obuf": Obuf, "OTbuf": OTbuf,
            "xT_loc": np.ascontiguousarray(xT_bw[:, cols]),
        })

    meta = dict(N=N, NW=NW, w_pc=w_pc, n_tiles=n_tiles, t_rows=t_rows, h0=h0,
                blos=tuple(tuple(b) for b in blos),
                bhis=tuple(tuple(b) for b in bhis))
    return meta, shared, percore, perm


def kernel(**inputs):
    global _LAST_RESULT
    from concourse.bass_utils import run_bass_kernel_spmd

    ncores = 8
    meta, shared, percore, perm = prep_inputs(inputs, ncores)
    import os
    key = (meta["N"], meta["w_pc"], meta["blos"], meta["bhis"],
           os.environ.get("K_ABLATE", ""), os.environ.get("K_WINCAP", ""),
           os.environ.get("K_TILECAP", ""))
    if key not in _CACHE:
        _CACHE[key] = build_program(
            meta["n_tiles"], meta["t_rows"], meta["w_pc"], meta["h0"],
            [list(b) for b in meta["blos"]], [list(b) for b in meta["bhis"]],
            ncores)
    nc = _CACHE[key]
    in_maps = [dict(shared, **percore[c]) for c in range(ncores)]
    res = run_bass_kernel_spmd(nc, in_maps, core_ids=list(range(ncores)),
                               **_RUN_KWARGS)
    _LAST_RESULT = res
    N, w_pc = meta["N"], meta["w_pc"]
    out = np.zeros((meta["NW"] * P, D), np.float32)
    for c in range(ncores):
        yc = res.results[c]["y"]
        for j in range(w_pc):
            g = int(perm[c, j])
            out[g * P:(g + 1) * P] = yc[j * P:(j + 1) * P]
    return np.ascontiguousarray(out[:N])


# revision 20
# speedup vs baseline: 1.0921x; 1.0921x over previous
"""Trainium2 Bass kernel for 5-relation GAT (nn_GAT_76716705841462). v2.

Strategy: destination-sharded, collective-free, bf16, host-built one-hots.
  * Host prep: 128-dst windows dealt across 8 cores by sorted edge count
    (equalizes per-slot block counts; SPMD program shared across cores).
    Per (window, rel): non-self-loop edges split lo (src<32768) / hi
    (src>=17280) for int16 gather indices; block counts = max over cores.
    Host precomputes one-hot O [slot e, dst n] and OT [dst n, slot e]
    (bf16); pad slots are zero rows so no masking is needed on device.
  * Phase A (replicated): node table T[n] = [h0|1|h1|1|as(2)|ad(2)|pad]
    (384 bf16 = 768 B rows) via bf16 matmul; plus per-core Twin table
    (same layout) for the core's dealt windows from xT_loc.
  * Phase B per window: 2 dma_gathers (lo/hi, rels concat) of 768-B rows;
    a_dst per edge slot via one tiny PE matmul per block (OT_b.T @ ad_win);
    expl = exp(leakyrelu(a_src + a_dst)) on [128, 2*nblk] tiles; weighted
    one-hot wt_bh = O_b * expl_bh built on the Scalar engine (Copy
    activation, per-partition scale); TensorE accumulates wt.T @ [G_h|1]
    into ps_r [128, 258] over the rel's blocks. Self-loops are handled
    densely once per window (shared by all 5 rels: same logit, h row in
    Twin). Normalize per rel, accumulate, add 5*bias.
"""

import numpy as np
import ml_dtypes

import concourse.bacc as bacc
import concourse.bass as bass
import concourse.mybir as mybir
import concourse.tile as tile
from concourse.library_config import mlp

P = 128
H = 2
C = 128
D = 256
R = 5
TW = 384          # table row width in bf16 elements (768 B, %256 == 0)
A_OFF = 258       # a_src at 258:260, a_dst at 260:262
LOW_CAP = 32768
NEG = 0.2
EPS = 1e-16

f32 = mybir.dt.float32
bf16 = mybir.dt.bfloat16
i16 = mybir.dt.int16
BF = ml_dtypes.bfloat16

_CACHE = {}
_RUN_KWARGS = {}
_LAST_RESULT = None


def build_program(n_tiles, t_rows, w_pc, h0, blos, bhis, num_devices):
    """blos/bhis: per window j, list of R ints (lo/hi block counts)."""
    import os
    ablate = set(os.environ.get("K_ABLATE", "").split(","))
    wincap = int(os.environ.get("K_WINCAP", 10**9))
    tilecap = int(os.environ.get("K_TILECAP", 10**9))
    nc = bacc.Bacc("TRN2", target_bir_lowering=False, debug=False,
                   num_devices=num_devices)

    nw_p = n_tiles * P
    xT = nc.dram_tensor("xT", [D, nw_p], bf16, kind="ExternalInput")
    xT_loc = nc.dram_tensor("xT_loc", [D, w_pc * P], bf16,
                            kind="ExternalInput")
    Wsrc = nc.dram_tensor("Wsrc", [D, D], f32, kind="ExternalInput")
    Wdst = nc.dram_tensor("Wdst", [D, D], f32, kind="ExternalInput")
    atts = nc.dram_tensor("atts", [1, D], f32, kind="ExternalInput")
    attd = nc.dram_tensor("attd", [1, D], f32, kind="ExternalInput")
    bias_in = nc.dram_tensor("bias_in", [1, D], f32, kind="ExternalInput")

    nblo = [sum(b) for b in blos]
    nbhi = [sum(b) for b in bhis]
    nblk = [a + b for a, b in zip(nblo, nbhi)]
    lo_cols = [n * P // 16 for n in nblo]
    hi_cols = [n * P // 16 for n in nbhi]
    lo_off = np.concatenate([[0], np.cumsum(lo_cols)]).astype(int)
    hi_off = np.concatenate([[0], np.cumsum(hi_cols)]).astype(int)
    ob_off = np.concatenate([[0], np.cumsum([n * P for n in nblk])]).astype(int)

    loidx = nc.dram_tensor("loidx", [P, int(lo_off[-1])], i16,
                           kind="ExternalInput")
    hiidx = nc.dram_tensor("hiidx", [P, int(hi_off[-1])], i16,
                           kind="ExternalInput")
    Obuf = nc.dram_tensor("Obuf", [P, int(ob_off[-1])], bf16,
                          kind="ExternalInput")
    OTbuf = nc.dram_tensor("OTbuf", [P, int(ob_off[-1])], bf16,
                           kind="ExternalInput")
    y = nc.dram_tensor("y", [w_pc * P, D], f32, kind="ExternalOutput")

    T = nc.dram_tensor("T", [t_rows, TW], bf16)
    Twin = nc.dram_tensor("Twin", [w_pc * P, TW], bf16)

    # ---- TileContext 1: table build ----
    with tile.TileContext(nc) as tc:
        with (
            tc.tile_pool(name="setup", bufs=1) as su,
            tc.tile_pool(name="ps_su", bufs=1, space="PSUM") as psu,
        ):
            ws_h = [su.tile([P, D], f32, name=f"ws_h{k}", tag=f"ws_h{k}")
                    for k in range(2)]
            wd_h = [su.tile([P, D], f32, name=f"wd_h{k}", tag=f"wd_h{k}")
                    for k in range(2)]
            for k in range(2):
                nc.sync.dma_start(ws_h[k][:], Wsrc[k * P:(k + 1) * P, :])
                nc.sync.dma_start(wd_h[k][:], Wdst[k * P:(k + 1) * P, :])
            ones1 = su.tile([1, P], f32, tag="ones1")
            nc.vector.memset(ones1[:], 1.0)
            atts_sb = su.tile([1, D], f32, tag="atts_sb")
            attd_sb = su.tile([1, D], f32, tag="attd_sb")
            nc.sync.dma_start(atts_sb[:], atts[:])
            nc.sync.dma_start(attd_sb[:], attd[:])
            atts_bc = su.tile([P, D], f32, tag="atts_bc")
            attd_bc = su.tile([P, D], f32, tag="attd_bc")
            for row_sb, bc in ((atts_sb, atts_bc), (attd_sb, attd_bc)):
                ps_bc = psu.tile([P, D], f32, name="ps_bc", tag="ps_bc",
                                 bufs=2)
                nc.tensor.matmul(out=ps_bc[:], lhsT=ones1[:], rhs=row_sb[:],
                                 start=True, stop=True)
                nc.vector.tensor_copy(bc[:], ps_bc[:])

            rhs_kf = [su.tile([P, TW], f32, name=f"rhs_kf{k}", tag=f"rhs_kf{k}")
                      for k in range(2)]
            rhs_k = [su.tile([P, TW], bf16, name=f"rhs_k{k}", tag=f"rhs_k{k}")
                     for k in range(2)]
            for k in range(2):
                rk = rhs_kf[k]
                nc.vector.memset(rk[:], 0.0)
                nc.vector.tensor_copy(rk[:, 0:C], ws_h[k][:, 0:C])
                nc.vector.tensor_copy(rk[:, C + 1:2 * C + 1], ws_h[k][:, C:D])
                for h in range(H):
                    for src_w, src_bc, col in (
                        (ws_h[k], atts_bc, A_OFF + h),
                        (wd_h[k], attd_bc, A_OFF + 2 + h),
                    ):
                        scratch = su.tile([P, C], f32, name="vscr",
                                          tag="vscr", bufs=2)
                        nc.vector.tensor_tensor(
                            out=scratch[:],
                            in0=src_w[:, h * C:(h + 1) * C],
                            in1=src_bc[:, h * C:(h + 1) * C],
                            op=mybir.AluOpType.mult)
                        nc.vector.tensor_reduce(
                            out=rk[:, col:col + 1], in_=scratch[:],
                            axis=mybir.AxisListType.X,
                            op=mybir.AluOpType.add)
                nc.vector.tensor_copy(rhs_k[k][:], rk[:])

            with (
                tc.tile_pool(name="sb_tbl", bufs=3) as stp,
                tc.tile_pool(name="ps_tbl", bufs=4, space="PSUM") as ptp,
            ):
                # pair-tile batching: 2 node tiles per iteration; input loads
                # dispatched from the scalar engine's HWDGE ring, the table
                # write as a single two-chunk DMA from the sync ring.
                t_list = (list(range(min(n_tiles, tilecap)))
                          + list(range(n_tiles, n_tiles + w_pc)))
                groups = []
                i = 0
                while i < len(t_list):
                    if (i + 1 < len(t_list)
                            and t_list[i + 1] == t_list[i] + 1
                            and (t_list[i] < n_tiles) == (t_list[i + 1] < n_tiles)):
                        groups.append((t_list[i], 2))
                        i += 2
                    else:
                        groups.append((t_list[i], 1))
                        i += 1
                for t, gn in groups:
                    if t < n_tiles:
                        src, dst, row0 = xT, T, t * P
                    else:
                        src, dst, row0 = xT_loc, Twin, (t - n_tiles) * P
                    xk0 = stp.tile([P, gn * P], bf16, name="xk0", tag="xk0")
                    xk1 = stp.tile([P, gn * P], bf16, name="xk1", tag="xk1")
                    nc.scalar.dma_start(xk0[:], src[0:P, row0:row0 + gn * P])
                    nc.scalar.dma_start(xk1[:], src[P:D, row0:row0 + gn * P])
                    stg = stp.tile([P, gn * TW], bf16, name="stg", tag="stg")
                    for u in range(gn):
                        ps_t = ptp.tile([P, TW], f32, name="ps_t", tag="ps_t")
                        nc.tensor.matmul(
                            out=ps_t[:], lhsT=xk0[:, u * P:(u + 1) * P],
                            rhs=rhs_k[0][:], start=True, stop=False)
                        nc.tensor.matmul(
                            out=ps_t[:], lhsT=xk1[:, u * P:(u + 1) * P],
                            rhs=rhs_k[1][:], start=False, stop=True)
                        nc.vector.tensor_copy(
                            stg[:, u * TW:(u + 1) * TW], ps_t[:])
                        nc.vector.memset(
                            stg[:, u * TW + C:u * TW + C + 1], 1.0)
                        nc.vector.memset(
                            stg[:, u * TW + 2 * C + 1:u * TW + 2 * C + 2], 1.0)
                    nc.sync.dma_start(
                        dst[row0:row0 + gn * P, :].rearrange(
                            "(u p) c -> p u c", u=gn),
                        stg[:].rearrange("p (u c) -> p u c", u=gn))

    # ---- TileContext 2: attention + aggregation ----
    with tile.TileContext(nc) as tc:
        with (
            tc.tile_pool(name="su2", bufs=1) as su,
            tc.tile_pool(name="ps_su2", bufs=1, space="PSUM") as psu,
            tc.tile_pool(name="sb_g", bufs=3) as sgp,
            tc.tile_pool(name="sb_o", bufs=3) as sop_,
            tc.tile_pool(name="sb_idx", bufs=3) as sip,
            tc.tile_pool(name="sb_wt", bufs=4) as swp,
            tc.tile_pool(name="sb_sm", bufs=3) as ssp,
            tc.tile_pool(name="sb_out", bufs=2) as sout,
            tc.tile_pool(name="ps_mm", bufs=2, space="PSUM") as pmp,
            tc.tile_pool(name="ps_ad", bufs=2, space="PSUM") as pap,
        ):
            nc.gpsimd.load_library(mlp)
            bias_sb = su.tile([1, D], f32, tag="bias_sb")
            nc.sync.dma_start(bias_sb[:], bias_in[:])
            ones1 = su.tile([1, P], f32, tag="ones1b")
            nc.vector.memset(ones1[:], 1.0)
            bias5 = su.tile([P, D], f32, tag="bias5")
            ps_bc = psu.tile([P, D], f32, tag="ps_bc2")
            nc.tensor.matmul(out=ps_bc[:], lhsT=ones1[:], rhs=bias_sb[:],
                             start=True, stop=True)
            nc.vector.tensor_scalar_mul(bias5[:], ps_bc[:], float(R))

            for j in range(min(w_pc, wincap)):
                rows = slice(j * P, (j + 1) * P)
                nb, nl, nh = nblk[j], nblo[j], nbhi[j]
                lo_b0 = np.concatenate([[0], np.cumsum(blos[j])]).astype(int)
                hi_b0 = np.concatenate([[0], np.cumsum(bhis[j])]).astype(int)

                twin = sgp.tile([P, TW], bf16, name="twin", tag="twin")
                nc.sync.dma_start(twin[:], Twin[rows, :])
                lo_t = sip.tile([P, max(lo_cols[j], 16)], i16, name="lo_t",
                                tag="lo_t")
                hi_t = sip.tile([P, max(hi_cols[j], 16)], i16, name="hi_t",
                                tag="hi_t")
                if lo_cols[j]:
                    nc.sync.dma_start(
                        lo_t[:, :lo_cols[j]],
                        loidx[:, int(lo_off[j]):int(lo_off[j + 1])])
                if hi_cols[j]:
                    nc.sync.dma_start(
                        hi_t[:, :hi_cols[j]],
                        hiidx[:, int(hi_off[j]):int(hi_off[j + 1])])
                Ot = sop_.tile([P, nb * P], bf16, name="Ot", tag="Ot")
                OTt = sop_.tile([P, nb * P], bf16, name="OTt", tag="OTt")
                nc.scalar.dma_start(
                    Ot[:], Obuf[:, int(ob_off[j]):int(ob_off[j + 1])])
                nc.scalar.dma_start(
                    OTt[:], OTbuf[:, int(ob_off[j]):int(ob_off[j + 1])])

                G = sgp.tile([P, nb * TW], bf16, name="G", tag="G")
                if "nogather" in ablate:
                    nc.vector.memset(G[:], 0.25)
                else:
                    # dma_gather is capped at 1024 indices (8 blocks) per call
                    for b0 in range(0, nl, 8):
                        bn = min(8, nl - b0)
                        nc.gpsimd.dma_gather(
                            out_ap=G[:, b0 * TW:(b0 + bn) * TW].rearrange(
                                "p (b e) -> p b e", e=TW),
                            in_ap=T[0:LOW_CAP, :],
                            idxs_ap=lo_t[:, b0 * 8:(b0 + bn) * 8],
                            num_idxs=bn * P, num_idxs_reg=bn * P,
                            elem_size=TW)
                    for b0 in range(0, nh, 8):
                        bn = min(8, nh - b0)
                        nc.gpsimd.dma_gather(
                            out_ap=G[:, (nl + b0) * TW:(nl + b0 + bn) * TW]
                                .rearrange("p (b e) -> p b e", e=TW),
                            in_ap=T[h0:t_rows, :],
                            idxs_ap=hi_t[:, b0 * 8:(b0 + bn) * 8],
                            num_idxs=bn * P, num_idxs_reg=bn * P,
                            elem_size=TW)

                # a_dst per edge slot: per block, [128e, 2] = OT_b.T @ ad_win
                ps_ad = pap.tile([P, 2 * nb], f32, name="ps_ad", tag="ps_ad")
                if "noad" in ablate:
                    nc.vector.memset(ps_ad[:], 0.0)
                else:
                    for b in range(nb):
                        nc.tensor.matmul(
                            out=ps_ad[:, 2 * b:2 * b + 2],
                            lhsT=OTt[:, b * P:(b + 1) * P],
                            rhs=twin[:, A_OFF + 2:A_OFF + 4],
                            start=True, stop=True)

                # asum[e, (b h)] = a_src(from G) + a_dst(ps_ad)
                asum = ssp.tile([P, 2 * nb], f32, name="asum", tag="asum")
                nc.vector.tensor_tensor(
                    out=asum[:].rearrange("p (b h) -> p b h", h=2),
                    in0=G[:].rearrange("p (b e) -> p b e", e=TW)
                        [:, :, A_OFF:A_OFF + 2],
                    in1=ps_ad[:].rearrange("p (b h) -> p b h", h=2),
                    op=mybir.AluOpType.add)
                lrl = ssp.tile([P, 2 * nb], f32, name="lrl", tag="lrl")
                nc.vector.scalar_tensor_tensor(
                    out=lrl[:], in0=asum[:], scalar=NEG, in1=asum[:],
                    op0=mybir.AluOpType.mult, op1=mybir.AluOpType.max)
                expl = ssp.tile([P, 2 * nb], f32, name="expl", tag="expl")
                nc.scalar.activation(expl[:], lrl[:],
                                     mybir.ActivationFunctionType.Exp)

                # self-loop terms (shared across rels)
                aslf = ssp.tile([P, 2], f32, name="aslf", tag="aslf")
                nc.vector.tensor_tensor(
                    out=aslf[:], in0=twin[:, A_OFF:A_OFF + 2],
                    in1=twin[:, A_OFF + 2:A_OFF + 4], op=mybir.AluOpType.add)
                lslf = ssp.tile([P, 2], f32, name="lslf", tag="lslf")
                nc.vector.scalar_tensor_tensor(
                    out=lslf[:], in0=aslf[:], scalar=NEG, in1=aslf[:],
                    op0=mybir.AluOpType.mult, op1=mybir.AluOpType.max)
                esl = ssp.tile([P, 2], f32, name="esl", tag="esl")
                nc.scalar.activation(esl[:], lslf[:],
                                     mybir.ActivationFunctionType.Exp)
                Cslf = ssp.tile([P, 2 * C], f32, name="Cslf", tag="Cslf")
                for h in range(H):
                    nc.vector.tensor_scalar_mul(
                        Cslf[:, h * C:(h + 1) * C],
                        twin[:, h * (C + 1):h * (C + 1) + C],
                        esl[:, h:h + 1])

                # Gs[b]: G rows scaled by expl per head: [h0|1]*e_h0, [h1|1]*e_h1
                # one DVE op per 8-block chunk (stride-0 broadcast on expl)
                QW = 2 * (C + 1)
                Gs = sgp.tile([P, nb * QW], bf16, name="Gs", tag="Gs",
                              bufs=2)
                if "nogs" in ablate:
                    nc.vector.memset(Gs[:], 0.5)
                else:
                    for b0 in range(0, nb, 8):
                        bn = min(8, nb - b0)
                        nc.vector.tensor_tensor(
                            out=Gs[:, b0 * QW:(b0 + bn) * QW].rearrange(
                                "p (b h q) -> p b h q", h=2, q=C + 1),
                            in0=G[:].rearrange("p (b e) -> p b e", e=TW)
                                [:, b0:b0 + bn, 0:QW].rearrange(
                                "p b (h q) -> p b h q", q=C + 1),
                            in1=expl[:, 2 * b0:2 * (b0 + bn)].rearrange(
                                "p (b h) -> p b h ()", h=2).broadcast_to(
                                [P, bn, 2, C + 1]),
                            op=mybir.AluOpType.mult)

                outacc = sout.tile([P, D], f32, name="outacc", tag="outacc")

                for r in range(R):
                    blk = ([b for b in range(lo_b0[r], lo_b0[r + 1])]
                           + [nl + b for b in range(hi_b0[r], hi_b0[r + 1])])
                    ps = pmp.tile([P, 2 * (C + 1)], f32, name="ps", tag="ps")
                    if "nomm" in ablate:
                        nc.vector.memset(ps[:], 1.0)
                    else:
                        for h in range(H):
                            for bi, b in enumerate(blk):
                                nc.tensor.matmul(
                                    out=ps[:, h * (C + 1):(h + 1) * (C + 1)],
                                    lhsT=Ot[:, b * P:(b + 1) * P],
                                    rhs=Gs[:, b * QW + h * (C + 1):
                                           b * QW + (h + 1) * (C + 1)],
                                    start=(bi == 0), stop=(bi == len(blk) - 1))
                    den = ssp.tile([P, 2], f32, name="den", tag="den")
                    nc.vector.scalar_tensor_tensor(
                        out=den[:],
                        in0=ps[:].rearrange("p (h q) -> p h q", q=C + 1)
                            [:, :, C:C + 1].rearrange("p h o -> p (h o)"),
                        scalar=EPS, in1=esl[:],
                        op0=mybir.AluOpType.add, op1=mybir.AluOpType.add)
                    recip = ssp.tile([P, 2], f32, name="recip", tag="recip")
                    nc.vector.reciprocal(recip[:], den[:])
                    num = ssp.tile([P, 2 * C], f32, name="num", tag="num")
                    nc.vector.tensor_tensor(
                        out=num[:].rearrange("p (h c) -> p h c", h=2),
                        in0=ps[:].rearrange("p (h q) -> p h q", q=C + 1)
                            [:, :, 0:C],
                        in1=Cslf[:].rearrange("p (h c) -> p h c", h=2),
                        op=mybir.AluOpType.add)
                    for h in range(H):
                        osl = outacc[:, h * C:(h + 1) * C]
                        nsl = num[:, h * C:(h + 1) * C]
                        if r == 0:
                            nc.vector.tensor_scalar_mul(
                                osl, nsl, recip[:, h:h + 1])
                        else:
                            nc.vector.scalar_tensor_tensor(
                                out=osl, in0=nsl, scalar=recip[:, h:h + 1],
                                in1=osl, op0=mybir.AluOpType.mult,
                                op1=mybir.AluOpType.add)
                nc.vector.tensor_tensor(out=outacc[:], in0=outacc[:],
                                        in1=bias5[:], op=mybir.AluOpType.add)
                nc.sync.dma_start(y[rows, :], outacc[:])

    nc.finalize()
    return nc


def _wrap16(vals):
    """[n] int array -> 16-partition-wrapped [128, n//16] int16 (replicated)."""
    n = len(vals)
    assert n % 16 == 0
    a = np.asarray(vals, np.int16).reshape(n // 16, 16).T
    return np.tile(a, (8, 1))


def prep_inputs(inputs, ncores):
    x = np.asarray(inputs["x"], dtype=np.float32)
    N = x.shape[0]
    nw_real = -(-N // P)
    NW = -(-nw_real // ncores) * ncores
    w_pc = NW // ncores
    n_tiles = nw_real
    t_rows = n_tiles * P
    h0 = t_rows - LOW_CAP

    rels = ["parent", "child", "precede", "follow", "peer"]
    ebuckets = [[None] * NW for _ in range(R)]
    totals = np.zeros(NW, np.int64)
    for r, rn in enumerate(rels):
        ei = np.asarray(inputs[f"edge_index_{rn}"])
        src = ei[0].astype(np.int64)
        dst = ei[1].astype(np.int64)
        order = np.argsort(dst, kind="stable")
        src, dst = src[order], dst[order]
        w_of = dst // P
        cnt = np.bincount(w_of, minlength=NW)
        starts = np.zeros(NW + 1, np.int64)
        np.cumsum(cnt, out=starts[1:])
        for w in range(NW):
            s, e = starts[w], starts[w + 1]
            ebuckets[r][w] = (src[s:e], dst[s:e] - w * P)
            totals[w] += e - s

    order = np.argsort(-totals, kind="stable")
    perm = np.zeros((ncores, w_pc), np.int64)
    for j in range(w_pc):
        grp = order[j * ncores:(j + 1) * ncores]
        if j % 2:
            grp = grp[::-1]
        perm[:, j] = grp

    blos, bhis = [], []
    asn = {}
    for j in range(w_pc):
        blo_j, bhi_j = [], []
        for r in range(R):
            must_lo = np.zeros(ncores, np.int64)
            must_hi = np.zeros(ncores, np.int64)
            tot = np.zeros(ncores, np.int64)
            for c in range(ncores):
                src, _ = ebuckets[r][perm[c, j]]
                must_lo[c] = int((src < h0).sum())
                must_hi[c] = int((src >= LOW_CAP).sum())
                tot[c] = len(src)
            BT = max(1, int(-(-tot.max() // P)))
            B1 = int(-(-must_lo.max() // P))
            B2 = BT - B1
            if B2 * P < must_hi.max():
                B2 = int(-(-must_hi.max() // P))
                B1 = BT - B2
                if B1 * P < must_lo.max():
                    BT += 1
                    B1 = BT - B2
            assert B1 * P >= must_lo.max() and B2 * P >= must_hi.max(), (
                j, r, B1, B2, must_lo.max(), must_hi.max())
            blo_j.append(B1)
            bhi_j.append(B2)
            for c in range(ncores):
                src, dl = ebuckets[r][perm[c, j]]
                is_lo = src < h0
                is_hi = src >= LOW_CAP
                flex = ~is_lo & ~is_hi
                n_lo = min(B1 * P, len(src) - int(is_hi.sum()))
                fi = np.flatnonzero(flex)
                n_flex_lo = n_lo - int(is_lo.sum())
                lo_sel = np.concatenate(
                    [np.flatnonzero(is_lo), fi[:n_flex_lo]])
                hi_sel = np.concatenate(
                    [np.flatnonzero(is_hi), fi[n_flex_lo:]])
                assert len(lo_sel) == n_lo
                assert len(hi_sel) == len(src) - n_lo <= B2 * P
                lo_src = np.zeros(B1 * P, np.int64)
                hi_src = np.zeros(B2 * P, np.int64)
                lo_dst = np.full(B1 * P, -1, np.int64)
                hi_dst = np.full(B2 * P, -1, np.int64)
                lo_src[:len(lo_sel)] = src[lo_sel]
                hi_src[:len(hi_sel)] = src[hi_sel] - h0
                lo_dst[:len(lo_sel)] = dl[lo_sel]
                hi_dst[:len(hi_sel)] = dl[hi_sel]
                asn[(c, j, r)] = (lo_src, hi_src, lo_dst, hi_dst)
        blos.append(blo_j)
        bhis.append(bhi_j)

    xTf = np.zeros((D, max(t_rows, NW * P)), np.float32)
    xTf[:, :N] = x.T
    xT_bw = xTf.astype(BF)
    xT_b = np.ascontiguousarray(xT_bw[:, :t_rows])

    shared = {
        "xT": xT_b,
        "Wsrc": np.ascontiguousarray(np.asarray(inputs["W_src"], np.float32)),
        "Wdst": np.ascontiguousarray(np.asarray(inputs["W_dst"], np.float32)),
        "atts": np.asarray(inputs["att_src"], np.float32).reshape(1, D).copy(),
        "attd": np.asarray(inputs["att_dst"], np.float32).reshape(1, D).copy(),
        "bias_in": np.asarray(inputs["bias"], np.float32).reshape(1, D).copy(),
    }

    nblo = [sum(b) for b in blos]
    nbhi = [sum(b) for b in bhis]
    nblk = [a + b for a, b in zip(nblo, nbhi)]
    lo_colsT = sum(n * P // 16 for n in nblo)
    hi_colsT = sum(n * P // 16 for n in nbhi)
    ob_colsT = sum(n * P for n in nblk)

    eye = np.eye(P, dtype=BF)
    percore = []
    for c in range(ncores):
        loidx = np.zeros((P, lo_colsT), np.int16)
        hiidx = np.zeros((P, hi_colsT), np.int16)
        Obuf = np.zeros((P, ob_colsT), BF)
        OTbuf = np.zeros((P, ob_colsT), BF)
        lo_p = hi_p = ob_p = 0
        for j in range(w_pc):
            lo_all = [asn[(c, j, r)][0] for r in range(R)]
            hi_all = [asn[(c, j, r)][1] for r in range(R)]
            dst_all = ([asn[(c, j, r)][2] for r in range(R)]
                       + [asn[(c, j, r)][3] for r in range(R)])
            lo_cat = np.concatenate(lo_all)
            hi_cat = np.concatenate(hi_all)
            dst_cat = np.concatenate(dst_all)
            nl, nh = len(lo_cat) // P, len(hi_cat) // P
            if nl:
                loidx[:, lo_p:lo_p + nl * P // 16] = _wrap16(lo_cat)
            if nh:
                hiidx[:, hi_p:hi_p + nh * P // 16] = _wrap16(hi_cat)
            lo_p += nl * P // 16
            hi_p += nh * P // 16
            nb = nl + nh
            dst_slots = dst_cat.reshape(nb, P)
            for b in range(nb):
                d = dst_slots[b]
                val = np.zeros((P, P), BF)
                valid = d >= 0
                val[valid, :] = eye[d[valid], :]
                Obuf[:, ob_p + b * P:ob_p + (b + 1) * P] = val
                OTbuf[:, ob_p + b * P:ob_p + (b + 1) * P] = val.T
            ob_p += nb * P
        cols = np.concatenate(
            [np.arange(perm[c, j] * P, (perm[c, j] + 1) * P)
             for j in range(w_pc)])
        percore.append({
            "loidx": loidx, "hiidx": hiidx, "Obuf": Obuf, "OTbuf": OTbuf,
            "xT_loc": np.ascontiguousarray(xT_bw[:, cols]),
        })

    meta = dict(N=N, NW=NW, w_pc=w_pc, n_tiles=n_tiles, t_rows=t_rows, h0=h0,
                blos=tuple(tuple(b) for b in blos),
                bhis=tuple(tuple(b) for b in bhis))
    return meta, shared, percore, perm


def kernel(**inputs):
    global _LAST_RESULT
    from concourse.bass_utils import run_bass_kernel_spmd

    ncores = 8
    meta, shared, percore, perm = prep_inputs(inputs, ncores)
    import os
    key = (meta["N"], meta["w_pc"], meta["blos"], meta["bhis"],
           os.environ.get("K_ABLATE", ""), os.environ.get("K_WINCAP", ""),
           os.environ.get("K_TILECAP", ""))
    if key not in _CACHE:
        _CACHE[key] = build_program(
            meta["n_tiles"], meta["t_rows"], meta["w_pc"], meta["h0"],
            [list(b) for b in meta["blos"]], [list(b) for b in meta["bhis"]],
            ncores)
    nc = _CACHE[key]
    in_maps = [dict(shared, **percore[c]) for c in range(ncores)]
    res = run_bass_kernel_spmd(nc, in_maps, core_ids=list(range(ncores)),
                               **_RUN_KWARGS)
    _LAST_RESULT = res
    N, w_pc = meta["N"], meta["w_pc"]
    out = np.zeros((meta["NW"] * P, D), np.float32)
    for c in range(ncores):
        yc = res.results[c]["y"]
        for j in range(w_pc):
            g = int(perm[c, j])
            out[g * P:(g + 1) * P] = yc[j * P:(j + 1) * P]
    return np.ascontiguousarray(out[:N])


# revision 24
# speedup vs baseline: 1.4734x; 1.3491x over previous
"""Trainium2 Bass kernel for 5-relation GAT (nn_GAT_76716705841462). v2.

Strategy: destination-sharded, collective-free, bf16, host-built one-hots.
  * Host prep: 128-dst windows dealt across 8 cores by sorted edge count
    (equalizes per-slot block counts; SPMD program shared across cores).
    Per (window, rel): non-self-loop edges split lo (src<32768) / hi
    (src>=17280) for int16 gather indices; block counts = max over cores.
    Host precomputes one-hot O [slot e, dst n] and OT [dst n, slot e]
    (bf16); pad slots are zero rows so no masking is needed on device.
  * Phase A (replicated): node table T[n] = [h0|1|h1|1|as(2)|ad(2)|pad]
    (384 bf16 = 768 B rows) via bf16 matmul; plus per-core Twin table
    (same layout) for the core's dealt windows from xT_loc.
  * Phase B per window: 2 dma_gathers (lo/hi, rels concat) of 768-B rows;
    a_dst per edge slot via one tiny PE matmul per block (OT_b.T @ ad_win);
    expl = exp(leakyrelu(a_src + a_dst)) on [128, 2*nblk] tiles; weighted
    one-hot wt_bh = O_b * expl_bh built on the Scalar engine (Copy
    activation, per-partition scale); TensorE accumulates wt.T @ [G_h|1]
    into ps_r [128, 258] over the rel's blocks. Self-loops are handled
    densely once per window (shared by all 5 rels: same logit, h row in
    Twin). Normalize per rel, accumulate, add 5*bias.
"""

import numpy as np
import ml_dtypes

import concourse.bacc as bacc
import concourse.bass as bass
import concourse.mybir as mybir
import concourse.tile as tile
from concourse.library_config import mlp

P = 128
H = 2
C = 128
D = 256
R = 5
TW = 384          # table row width in bf16 elements (768 B, %256 == 0)
A_OFF = 258       # a_src at 258:260, a_dst at 260:262
LOW_CAP = 32768
NEG = 0.2
EPS = 1e-16

f32 = mybir.dt.float32
bf16 = mybir.dt.bfloat16
i16 = mybir.dt.int16
BF = ml_dtypes.bfloat16

_CACHE = {}
_RUN_KWARGS = {}
_LAST_RESULT = None


def build_program(n_tiles, t_rows, w_pc, h0, blos, bhis, num_devices):
    """blos/bhis: per window j, list of R ints (lo/hi block counts)."""
    import os
    ablate = set(os.environ.get("K_ABLATE", "").split(","))
    wincap = int(os.environ.get("K_WINCAP", 10**9))
    tilecap = int(os.environ.get("K_TILECAP", 10**9))
    nc = bacc.Bacc("TRN2", target_bir_lowering=False, debug=False,
                   num_devices=num_devices, num_swdge_queues=4)

    nw_p = n_tiles * P
    xT = nc.dram_tensor("xT", [D, nw_p], bf16, kind="ExternalInput")
    xT_loc = nc.dram_tensor("xT_loc", [D, w_pc * P], bf16,
                            kind="ExternalInput")
    Wsrc = nc.dram_tensor("Wsrc", [D, D], f32, kind="ExternalInput")
    Wdst = nc.dram_tensor("Wdst", [D, D], f32, kind="ExternalInput")
    atts = nc.dram_tensor("atts", [1, D], f32, kind="ExternalInput")
    attd = nc.dram_tensor("attd", [1, D], f32, kind="ExternalInput")
    bias_in = nc.dram_tensor("bias_in", [1, D], f32, kind="ExternalInput")

    nblo = [sum(b) for b in blos]
    nbhi = [sum(b) for b in bhis]
    nblk = [a + b for a, b in zip(nblo, nbhi)]
    lo_cols = [n * P // 16 for n in nblo]
    hi_cols = [n * P // 16 for n in nbhi]
    lo_off = np.concatenate([[0], np.cumsum(lo_cols)]).astype(int)
    hi_off = np.concatenate([[0], np.cumsum(hi_cols)]).astype(int)
    ob_off = np.concatenate([[0], np.cumsum([n * P for n in nblk])]).astype(int)

    loidx = nc.dram_tensor("loidx", [P, int(lo_off[-1])], i16,
                           kind="ExternalInput")
    hiidx = nc.dram_tensor("hiidx", [P, int(hi_off[-1])], i16,
                           kind="ExternalInput")
    Obuf = nc.dram_tensor("Obuf", [P, int(ob_off[-1])], bf16,
                          kind="ExternalInput")
    OTbuf = nc.dram_tensor("OTbuf", [P, int(ob_off[-1])], bf16,
                           kind="ExternalInput")
    y = nc.dram_tensor("y", [w_pc * P, D], f32, kind="ExternalOutput")

    T = nc.dram_tensor("T", [t_rows, TW], bf16)
    Twin = nc.dram_tensor("Twin", [w_pc * P, TW], bf16)

    # ---- TileContext 1: table build ----
    with tile.TileContext(nc) as tc:
        with (
            tc.tile_pool(name="setup", bufs=1) as su,
            tc.tile_pool(name="ps_su", bufs=1, space="PSUM") as psu,
        ):
            ws_h = [su.tile([P, D], f32, name=f"ws_h{k}", tag=f"ws_h{k}")
                    for k in range(2)]
            wd_h = [su.tile([P, D], f32, name=f"wd_h{k}", tag=f"wd_h{k}")
                    for k in range(2)]
            for k in range(2):
                nc.sync.dma_start(ws_h[k][:], Wsrc[k * P:(k + 1) * P, :])
                nc.sync.dma_start(wd_h[k][:], Wdst[k * P:(k + 1) * P, :])
            ones1 = su.tile([1, P], f32, tag="ones1")
            nc.vector.memset(ones1[:], 1.0)
            atts_sb = su.tile([1, D], f32, tag="atts_sb")
            attd_sb = su.tile([1, D], f32, tag="attd_sb")
            nc.sync.dma_start(atts_sb[:], atts[:])
            nc.sync.dma_start(attd_sb[:], attd[:])
            atts_bc = su.tile([P, D], f32, tag="atts_bc")
            attd_bc = su.tile([P, D], f32, tag="attd_bc")
            for row_sb, bc in ((atts_sb, atts_bc), (attd_sb, attd_bc)):
                ps_bc = psu.tile([P, D], f32, name="ps_bc", tag="ps_bc",
                                 bufs=2)
                nc.tensor.matmul(out=ps_bc[:], lhsT=ones1[:], rhs=row_sb[:],
                                 start=True, stop=True)
                nc.vector.tensor_copy(bc[:], ps_bc[:])

            rhs_kf = [su.tile([P, TW], f32, name=f"rhs_kf{k}", tag=f"rhs_kf{k}")
                      for k in range(2)]
            rhs_k = [su.tile([P, TW], bf16, name=f"rhs_k{k}", tag=f"rhs_k{k}")
                     for k in range(2)]
            for k in range(2):
                rk = rhs_kf[k]
                nc.vector.memset(rk[:], 0.0)
                nc.vector.tensor_copy(rk[:, 0:C], ws_h[k][:, 0:C])
                nc.vector.tensor_copy(rk[:, C + 1:2 * C + 1], ws_h[k][:, C:D])
                for h in range(H):
                    for src_w, src_bc, col in (
                        (ws_h[k], atts_bc, A_OFF + h),
                        (wd_h[k], attd_bc, A_OFF + 2 + h),
                    ):
                        scratch = su.tile([P, C], f32, name="vscr",
                                          tag="vscr", bufs=2)
                        nc.vector.tensor_tensor(
                            out=scratch[:],
                            in0=src_w[:, h * C:(h + 1) * C],
                            in1=src_bc[:, h * C:(h + 1) * C],
                            op=mybir.AluOpType.mult)
                        nc.vector.tensor_reduce(
                            out=rk[:, col:col + 1], in_=scratch[:],
                            axis=mybir.AxisListType.X,
                            op=mybir.AluOpType.add)
                nc.vector.tensor_copy(rhs_k[k][:], rk[:])

            with (
                tc.tile_pool(name="sb_tbl", bufs=3) as stp,
                tc.tile_pool(name="ps_tbl", bufs=4, space="PSUM") as ptp,
            ):
                # pair-tile batching: 2 node tiles per iteration; input loads
                # dispatched from the scalar engine's HWDGE ring, the table
                # write as a single two-chunk DMA from the sync ring.
                t_list = (list(range(min(n_tiles, tilecap)))
                          + list(range(n_tiles, n_tiles + w_pc)))
                groups = []
                i = 0
                while i < len(t_list):
                    gn = 1
                    while (gn < 4 and i + gn < len(t_list)
                           and t_list[i + gn] == t_list[i] + gn
                           and (t_list[i] < n_tiles)
                           == (t_list[i + gn] < n_tiles)):
                        gn += 1
                    groups.append((t_list[i], gn))
                    i += gn
                for t, gn in groups:
                    if t < n_tiles:
                        src, dst, row0 = xT, T, t * P
                    else:
                        src, dst, row0 = xT_loc, Twin, (t - n_tiles) * P
                    xk0 = stp.tile([P, gn * P], bf16, name="xk0", tag="xk0")
                    xk1 = stp.tile([P, gn * P], bf16, name="xk1", tag="xk1")
                    nc.scalar.dma_start(xk0[:], src[0:P, row0:row0 + gn * P])
                    nc.sync.dma_start(xk1[:], src[P:D, row0:row0 + gn * P])
                    stg = stp.tile([P, gn * TW], bf16, name="stg", tag="stg")
                    for u in range(gn):
                        ps_t = ptp.tile([P, TW], f32, name="ps_t", tag="ps_t")
                        nc.tensor.matmul(
                            out=ps_t[:], lhsT=xk0[:, u * P:(u + 1) * P],
                            rhs=rhs_k[0][:], start=True, stop=False)
                        nc.tensor.matmul(
                            out=ps_t[:], lhsT=xk1[:, u * P:(u + 1) * P],
                            rhs=rhs_k[1][:], start=False, stop=True)
                        nc.vector.tensor_copy(
                            stg[:, u * TW:(u + 1) * TW], ps_t[:])
                        nc.vector.memset(
                            stg[:, u * TW + C:u * TW + C + 1], 1.0)
                        nc.vector.memset(
                            stg[:, u * TW + 2 * C + 1:u * TW + 2 * C + 2], 1.0)
                    nc.sync.dma_start(
                        dst[row0:row0 + gn * P, :].rearrange(
                            "(u p) c -> p u c", u=gn),
                        stg[:].rearrange("p (u c) -> p u c", u=gn))

    # ---- TileContext 2: attention + aggregation ----
    with tile.TileContext(nc) as tc:
        with (
            tc.tile_pool(name="su2", bufs=1) as su,
            tc.tile_pool(name="ps_su2", bufs=1, space="PSUM") as psu,
            tc.tile_pool(name="sb_g", bufs=3) as sgp,
            tc.tile_pool(name="sb_o", bufs=3) as sop_,
            tc.tile_pool(name="sb_idx", bufs=3) as sip,
            tc.tile_pool(name="sb_wt", bufs=4) as swp,
            tc.tile_pool(name="sb_sm", bufs=3) as ssp,
            tc.tile_pool(name="sb_out", bufs=2) as sout,
            tc.tile_pool(name="ps_mm", bufs=2, space="PSUM") as pmp,
            tc.tile_pool(name="ps_ad", bufs=2, space="PSUM") as pap,
        ):
            nc.gpsimd.load_library(mlp)
            bias_sb = su.tile([1, D], f32, tag="bias_sb")
            nc.sync.dma_start(bias_sb[:], bias_in[:])
            ones1 = su.tile([1, P], f32, tag="ones1b")
            nc.vector.memset(ones1[:], 1.0)
            bias5 = su.tile([P, D], f32, tag="bias5")
            ps_bc = psu.tile([P, D], f32, tag="ps_bc2")
            nc.tensor.matmul(out=ps_bc[:], lhsT=ones1[:], rhs=bias_sb[:],
                             start=True, stop=True)
            nc.vector.tensor_scalar_mul(bias5[:], ps_bc[:], float(R))
            qrr = [0]

            for j in range(min(w_pc, wincap)):
                rows = slice(j * P, (j + 1) * P)
                nb, nl, nh = nblk[j], nblo[j], nbhi[j]
                lo_b0 = np.concatenate([[0], np.cumsum(blos[j])]).astype(int)
                hi_b0 = np.concatenate([[0], np.cumsum(bhis[j])]).astype(int)

                twin = sgp.tile([P, TW], bf16, name="twin", tag="twin")
                nc.sync.dma_start(twin[:], Twin[rows, :])
                lo_t = sip.tile([P, max(lo_cols[j], 16)], i16, name="lo_t",
                                tag="lo_t")
                hi_t = sip.tile([P, max(hi_cols[j], 16)], i16, name="hi_t",
                                tag="hi_t")
                if lo_cols[j]:
                    nc.sync.dma_start(
                        lo_t[:, :lo_cols[j]],
                        loidx[:, int(lo_off[j]):int(lo_off[j + 1])])
                if hi_cols[j]:
                    nc.sync.dma_start(
                        hi_t[:, :hi_cols[j]],
                        hiidx[:, int(hi_off[j]):int(hi_off[j + 1])])
                Ot = sop_.tile([P, nb * P], bf16, name="Ot", tag="Ot")
                OTt = sop_.tile([P, nb * P], bf16, name="OTt", tag="OTt")
                nc.scalar.dma_start(
                    Ot[:], Obuf[:, int(ob_off[j]):int(ob_off[j + 1])])
                nc.scalar.dma_start(
                    OTt[:], OTbuf[:, int(ob_off[j]):int(ob_off[j + 1])])

                G = sgp.tile([P, nb * TW], bf16, name="G", tag="G")
                if "nogather" in ablate:
                    nc.vector.memset(G[:], 0.25)
                else:
                    # dma_gather is capped at 1024 indices (8 blocks) per
                    # call; round-robin the 4 SWDGE queues (desc-gen runs on
                    # a different Q7 cpu pair per queue -> ~3.3x concurrency)
                    for b0 in range(0, nl, 8):
                        bn = min(8, nl - b0)
                        nc.gpsimd.dma_gather(
                            out_ap=G[:, b0 * TW:(b0 + bn) * TW].rearrange(
                                "p (b e) -> p b e", e=TW),
                            in_ap=T[0:LOW_CAP, :],
                            idxs_ap=lo_t[:, b0 * 8:(b0 + bn) * 8],
                            num_idxs=bn * P, num_idxs_reg=bn * P,
                            elem_size=TW, queue_num=qrr[0] % 4)
                        qrr[0] += 1
                    for b0 in range(0, nh, 8):
                        bn = min(8, nh - b0)
                        nc.gpsimd.dma_gather(
                            out_ap=G[:, (nl + b0) * TW:(nl + b0 + bn) * TW]
                                .rearrange("p (b e) -> p b e", e=TW),
                            in_ap=T[h0:t_rows, :],
                            idxs_ap=hi_t[:, b0 * 8:(b0 + bn) * 8],
                            num_idxs=bn * P, num_idxs_reg=bn * P,
                            elem_size=TW, queue_num=qrr[0] % 4)
                        qrr[0] += 1

                # a_dst per edge slot: per block, [128e, 2] = OT_b.T @ ad_win
                ps_ad = pap.tile([P, 2 * nb], f32, name="ps_ad", tag="ps_ad")
                if "noad" in ablate:
                    nc.vector.memset(ps_ad[:], 0.0)
                else:
                    for b in range(nb):
                        nc.tensor.matmul(
                            out=ps_ad[:, 2 * b:2 * b + 2],
                            lhsT=OTt[:, b * P:(b + 1) * P],
                            rhs=twin[:, A_OFF + 2:A_OFF + 4],
                            start=True, stop=True)

                # asum[e, (b h)] = a_src(from G) + a_dst(ps_ad)
                asum = ssp.tile([P, 2 * nb], f32, name="asum", tag="asum")
                nc.vector.tensor_tensor(
                    out=asum[:].rearrange("p (b h) -> p b h", h=2),
                    in0=G[:].rearrange("p (b e) -> p b e", e=TW)
                        [:, :, A_OFF:A_OFF + 2],
                    in1=ps_ad[:].rearrange("p (b h) -> p b h", h=2),
                    op=mybir.AluOpType.add)
                lrl = ssp.tile([P, 2 * nb], f32, name="lrl", tag="lrl")
                nc.vector.scalar_tensor_tensor(
                    out=lrl[:], in0=asum[:], scalar=NEG, in1=asum[:],
                    op0=mybir.AluOpType.mult, op1=mybir.AluOpType.max)
                expl = ssp.tile([P, 2 * nb], f32, name="expl", tag="expl")
                nc.scalar.activation(expl[:], lrl[:],
                                     mybir.ActivationFunctionType.Exp)

                # self-loop terms (shared across rels)
                aslf = ssp.tile([P, 2], f32, name="aslf", tag="aslf")
                nc.vector.tensor_tensor(
                    out=aslf[:], in0=twin[:, A_OFF:A_OFF + 2],
                    in1=twin[:, A_OFF + 2:A_OFF + 4], op=mybir.AluOpType.add)
                lslf = ssp.tile([P, 2], f32, name="lslf", tag="lslf")
                nc.vector.scalar_tensor_tensor(
                    out=lslf[:], in0=aslf[:], scalar=NEG, in1=aslf[:],
                    op0=mybir.AluOpType.mult, op1=mybir.AluOpType.max)
                esl = ssp.tile([P, 2], f32, name="esl", tag="esl")
                nc.scalar.activation(esl[:], lslf[:],
                                     mybir.ActivationFunctionType.Exp)
                Cslf = ssp.tile([P, 2 * C], f32, name="Cslf", tag="Cslf")
                for h in range(H):
                    nc.vector.tensor_scalar_mul(
                        Cslf[:, h * C:(h + 1) * C],
                        twin[:, h * (C + 1):h * (C + 1) + C],
                        esl[:, h:h + 1])

                # Gs[b]: G rows scaled by expl per head: [h0|1]*e_h0, [h1|1]*e_h1
                # one DVE op per 8-block chunk (stride-0 broadcast on expl)
                QW = 2 * (C + 1)
                Gs = sgp.tile([P, nb * QW], bf16, name="Gs", tag="Gs",
                              bufs=2)
                if "nogs" in ablate:
                    nc.vector.memset(Gs[:], 0.5)
                else:
                    for b0 in range(0, nb, 8):
                        bn = min(8, nb - b0)
                        nc.vector.tensor_tensor(
                            out=Gs[:, b0 * QW:(b0 + bn) * QW].rearrange(
                                "p (b h q) -> p b h q", h=2, q=C + 1),
                            in0=G[:].rearrange("p (b e) -> p b e", e=TW)
                                [:, b0:b0 + bn, 0:QW].rearrange(
                                "p b (h q) -> p b h q", q=C + 1),
                            in1=expl[:, 2 * b0:2 * (b0 + bn)].rearrange(
                                "p (b h) -> p b h ()", h=2).broadcast_to(
                                [P, bn, 2, C + 1]),
                            op=mybir.AluOpType.mult)

                outacc = sout.tile([P, D], f32, name="outacc", tag="outacc")

                for r in range(R):
                    blk = ([b for b in range(lo_b0[r], lo_b0[r + 1])]
                           + [nl + b for b in range(hi_b0[r], hi_b0[r + 1])])
                    ps = pmp.tile([P, 2 * (C + 1)], f32, name="ps", tag="ps")
                    if "nomm" in ablate:
                        nc.vector.memset(ps[:], 1.0)
                    else:
                        for h in range(H):
                            for bi, b in enumerate(blk):
                                nc.tensor.matmul(
                                    out=ps[:, h * (C + 1):(h + 1) * (C + 1)],
                                    lhsT=Ot[:, b * P:(b + 1) * P],
                                    rhs=Gs[:, b * QW + h * (C + 1):
                                           b * QW + (h + 1) * (C + 1)],
                                    start=(bi == 0), stop=(bi == len(blk) - 1))
                    den = ssp.tile([P, 2], f32, name="den", tag="den")
                    nc.vector.scalar_tensor_tensor(
                        out=den[:],
                        in0=ps[:].rearrange("p (h q) -> p h q", q=C + 1)
                            [:, :, C:C + 1].rearrange("p h o -> p (h o)"),
                        scalar=EPS, in1=esl[:],
                        op0=mybir.AluOpType.add, op1=mybir.AluOpType.add)
                    recip = ssp.tile([P, 2], f32, name="recip", tag="recip")
                    nc.vector.reciprocal(recip[:], den[:])
                    num = ssp.tile([P, 2 * C], f32, name="num", tag="num")
                    nc.vector.tensor_tensor(
                        out=num[:].rearrange("p (h c) -> p h c", h=2),
                        in0=ps[:].rearrange("p (h q) -> p h q", q=C + 1)
                            [:, :, 0:C],
                        in1=Cslf[:].rearrange("p (h c) -> p h c", h=2),
                        op=mybir.AluOpType.add)
                    for h in range(H):
                        osl = outacc[:, h * C:(h + 1) * C]
                        nsl = num[:, h * C:(h + 1) * C]
                        if r == 0:
                            nc.vector.tensor_scalar_mul(
                                osl, nsl, recip[:, h:h + 1])
                        else:
                            nc.vector.scalar_tensor_tensor(
                                out=osl, in0=nsl, scalar=recip[:, h:h + 1],
                                in1=osl, op0=mybir.AluOpType.mult,
                                op1=mybir.AluOpType.add)
                nc.vector.tensor_tensor(out=outacc[:], in0=outacc[:],
                                        in1=bias5[:], op=mybir.AluOpType.add)
                nc.sync.dma_start(y[rows, :], outacc[:])

    nc.finalize()
    return nc


def _wrap16(vals):
    """[n] int array -> 16-partition-wrapped [128, n//16] int16 (replicated)."""
    n = len(vals)
    assert n % 16 == 0
    a = np.asarray(vals, np.int16).reshape(n // 16, 16).T
    return np.tile(a, (8, 1))


def prep_inputs(inputs, ncores):
    x = np.asarray(inputs["x"], dtype=np.float32)
    N = x.shape[0]
    nw_real = -(-N // P)
    NW = -(-nw_real // ncores) * ncores
    w_pc = NW // ncores
    n_tiles = nw_real
    t_rows = n_tiles * P
    h0 = t_rows - LOW_CAP

    rels = ["parent", "child", "precede", "follow", "peer"]
    ebuckets = [[None] * NW for _ in range(R)]
    totals = np.zeros(NW, np.int64)
    for r, rn in enumerate(rels):
        ei = np.asarray(inputs[f"edge_index_{rn}"])
        src = ei[0].astype(np.int64)
        dst = ei[1].astype(np.int64)
        order = np.argsort(dst, kind="stable")
        src, dst = src[order], dst[order]
        w_of = dst // P
        cnt = np.bincount(w_of, minlength=NW)
        starts = np.zeros(NW + 1, np.int64)
        np.cumsum(cnt, out=starts[1:])
        for w in range(NW):
            s, e = starts[w], starts[w + 1]
            ebuckets[r][w] = (src[s:e], dst[s:e] - w * P)
            totals[w] += e - s

    order = np.argsort(-totals, kind="stable")
    perm = np.zeros((ncores, w_pc), np.int64)
    for j in range(w_pc):
        grp = order[j * ncores:(j + 1) * ncores]
        if j % 2:
            grp = grp[::-1]
        perm[:, j] = grp

    blos, bhis = [], []
    asn = {}
    for j in range(w_pc):
        blo_j, bhi_j = [], []
        for r in range(R):
            must_lo = np.zeros(ncores, np.int64)
            must_hi = np.zeros(ncores, np.int64)
            tot = np.zeros(ncores, np.int64)
            for c in range(ncores):
                src, _ = ebuckets[r][perm[c, j]]
                must_lo[c] = int((src < h0).sum())
                must_hi[c] = int((src >= LOW_CAP).sum())
                tot[c] = len(src)
            BT = max(1, int(-(-tot.max() // P)))
            B1 = int(-(-must_lo.max() // P))
            B2 = BT - B1
            if B2 * P < must_hi.max():
                B2 = int(-(-must_hi.max() // P))
                B1 = BT - B2
                if B1 * P < must_lo.max():
                    BT += 1
                    B1 = BT - B2
            assert B1 * P >= must_lo.max() and B2 * P >= must_hi.max(), (
                j, r, B1, B2, must_lo.max(), must_hi.max())
            blo_j.append(B1)
            bhi_j.append(B2)
            for c in range(ncores):
                src, dl = ebuckets[r][perm[c, j]]
                is_lo = src < h0
                is_hi = src >= LOW_CAP
                flex = ~is_lo & ~is_hi
                n_lo = min(B1 * P, len(src) - int(is_hi.sum()))
                fi = np.flatnonzero(flex)
                n_flex_lo = n_lo - int(is_lo.sum())
                lo_sel = np.concatenate(
                    [np.flatnonzero(is_lo), fi[:n_flex_lo]])
                hi_sel = np.concatenate(
                    [np.flatnonzero(is_hi), fi[n_flex_lo:]])
                assert len(lo_sel) == n_lo
                assert len(hi_sel) == len(src) - n_lo <= B2 * P
                lo_src = np.zeros(B1 * P, np.int64)
                hi_src = np.zeros(B2 * P, np.int64)
                lo_dst = np.full(B1 * P, -1, np.int64)
                hi_dst = np.full(B2 * P, -1, np.int64)
                lo_src[:len(lo_sel)] = src[lo_sel]
                hi_src[:len(hi_sel)] = src[hi_sel] - h0
                lo_dst[:len(lo_sel)] = dl[lo_sel]
                hi_dst[:len(hi_sel)] = dl[hi_sel]
                asn[(c, j, r)] = (lo_src, hi_src, lo_dst, hi_dst)
        blos.append(blo_j)
        bhis.append(bhi_j)

    xTf = np.zeros((D, max(t_rows, NW * P)), np.float32)
    xTf[:, :N] = x.T
    xT_bw = xTf.astype(BF)
    xT_b = np.ascontiguousarray(xT_bw[:, :t_rows])

    shared = {
        "xT": xT_b,
        "Wsrc": np.ascontiguousarray(np.asarray(inputs["W_src"], np.float32)),
        "Wdst": np.ascontiguousarray(np.asarray(inputs["W_dst"], np.float32)),
        "atts": np.asarray(inputs["att_src"], np.float32).reshape(1, D).copy(),
        "attd": np.asarray(inputs["att_dst"], np.float32).reshape(1, D).copy(),
        "bias_in": np.asarray(inputs["bias"], np.float32).reshape(1, D).copy(),
    }

    nblo = [sum(b) for b in blos]
    nbhi = [sum(b) for b in bhis]
    nblk = [a + b for a, b in zip(nblo, nbhi)]
    lo_colsT = sum(n * P // 16 for n in nblo)
    hi_colsT = sum(n * P // 16 for n in nbhi)
    ob_colsT = sum(n * P for n in nblk)

    eye = np.eye(P, dtype=BF)
    percore = []
    for c in range(ncores):
        loidx = np.zeros((P, lo_colsT), np.int16)
        hiidx = np.zeros((P, hi_colsT), np.int16)
        Obuf = np.zeros((P, ob_colsT), BF)
        OTbuf = np.zeros((P, ob_colsT), BF)
        lo_p = hi_p = ob_p = 0
        for j in range(w_pc):
            lo_all = [asn[(c, j, r)][0] for r in range(R)]
            hi_all = [asn[(c, j, r)][1] for r in range(R)]
            dst_all = ([asn[(c, j, r)][2] for r in range(R)]
                       + [asn[(c, j, r)][3] for r in range(R)])
            lo_cat = np.concatenate(lo_all)
            hi_cat = np.concatenate(hi_all)
            dst_cat = np.concatenate(dst_all)
            nl, nh = len(lo_cat) // P, len(hi_cat) // P
            if nl:
                loidx[:, lo_p:lo_p + nl * P // 16] = _wrap16(lo_cat)
            if nh:
                hiidx[:, hi_p:hi_p + nh * P // 16] = _wrap16(hi_cat)
            lo_p += nl * P // 16
            hi_p += nh * P // 16
            nb = nl + nh
            dst_slots = dst_cat.reshape(nb, P)
            for b in range(nb):
                d = dst_slots[b]
                val = np.zeros((P, P), BF)
                valid = d >= 0
                val[valid, :] = eye[d[valid], :]
                Obuf[:, ob_p + b * P:ob_p + (b + 1) * P] = val
                OTbuf[:, ob_p + b * P:ob_p + (b + 1) * P] = val.T
            ob_p += nb * P
        cols = np.concatenate(
            [np.arange(perm[c, j] * P, (perm[c, j] + 1) * P)
             for j in range(w_pc)])
        percore.append({
            "loidx": loidx, "hiidx": hiidx, "Obuf": Obuf, "OTbuf": OTbuf,
            "xT_loc": np.ascontiguousarray(xT_bw[:, cols]),
        })

    meta = dict(N=N, NW=NW, w_pc=w_pc, n_tiles=n_tiles, t_rows=t_rows, h0=h0,
                blos=tuple(tuple(b) for b in blos),
                bhis=tuple(tuple(b) for b in bhis))
    return meta, shared, percore, perm


def kernel(**inputs):
    global _LAST_RESULT
    from concourse.bass_utils import run_bass_kernel_spmd

    ncores = 8
    meta, shared, percore, perm = prep_inputs(inputs, ncores)
    import os
    key = (meta["N"], meta["w_pc"], meta["blos"], meta["bhis"],
           os.environ.get("K_ABLATE", ""), os.environ.get("K_WINCAP", ""),
           os.environ.get("K_TILECAP", ""))
    if key not in _CACHE:
        _CACHE[key] = build_program(
            meta["n_tiles"], meta["t_rows"], meta["w_pc"], meta["h0"],
            [list(b) for b in meta["blos"]], [list(b) for b in meta["bhis"]],
            ncores)
    nc = _CACHE[key]
    in_maps = [dict(shared, **percore[c]) for c in range(ncores)]
    res = run_bass_kernel_spmd(nc, in_maps, core_ids=list(range(ncores)),
                               **_RUN_KWARGS)
    _LAST_RESULT = res
    N, w_pc = meta["N"], meta["w_pc"]
    out = np.zeros((meta["NW"] * P, D), np.float32)
    for c in range(ncores):
        yc = res.results[c]["y"]
        for j in range(w_pc):
            g = int(perm[c, j])
            out[g * P:(g + 1) * P] = yc[j * P:(j + 1) * P]
    return np.ascontiguousarray(out[:N])
